# revision 21
# baseline (speedup 1.0000x reference)
"""Distributed Trainium2 kernel for the sparse-attention module.

Shapes (hardcoded): B=2, N=2048, D=512, H=8, HD=64, MAX_LEN=1000, BAND=3.
Sharding: 16 (batch, head) pairs over 8 cores -> each core owns one batch b
and one head-pair hp (2 heads = 128 cols of the projected D dimension).

Per core (all matmuls bf16, f32 accumulation):
  qpT = (Wq_cols/8)^T @ q[b]^T   [128, 2048]   (1/8 score scale folded in)
  kpT = Wk_cols^T @ k[b]^T       [128, 2048]
  v   = Wv2^T @ V_emb_cols       [2048, 128]   (s on partitions, 16 tiles)
  per head h (rows 64h:64h+64 of qpT/kpT), two independent pipelines:
   A: ST scores (j on part.) -> band -> exp -> U bf16; ov += v_h^T @ U (psum
      bursts of 4 j-tiles, reduced into ovacc bf16 in SBUF)
   B: S scores (i on part.) -> band -> exp(accum_out=rowsum) -> recip ->
      normalize on VectorE -> DMA attn out
  pout = sum_h (ovacc_h^T @ Wout_h) * recip_h[i]  (host sums across cores)
"""

import sys
import os
import numpy as np

sys.path.insert(0, "/opt/trn_rl_repo")

import ml_dtypes

BF16 = ml_dtypes.bfloat16
FP8 = ml_dtypes.float8_e4m3

B, N, D, H = 2, 2048, 512, 8
HD = D // H
MAX_LEN = 1000
MPAD = 1024  # V_emb/Wv2 contraction dim padded to 8*128
BAND = 3
NT = N // 128  # 16 row tiles
NC = N // 512  # 4 free chunks

_cache = {}


def _build_nc():
    import concourse.bacc as bacc
    import concourse.tile as tile
    from concourse import mybir

    f32 = mybir.dt.float32
    bf16 = mybir.dt.bfloat16
    fp8 = mybir.dt.float8e4
    AF = mybir.ActivationFunctionType
    ALU = mybir.AluOpType

    nc = bacc.Bacc("TRN2", target_bir_lowering=False, num_devices=8)

    # ---- DRAM parameters (per-core shards; host preps layouts) ----
    d_qT = nc.declare_dram_parameter("qT", [128, 4, N], bf16, isOutput=False)
    d_kT = nc.declare_dram_parameter("kT", [128, 4, N], bf16, isOutput=False)
    d_wq = nc.declare_dram_parameter("wq", [128, 4, 128], bf16, isOutput=False)
    d_wk = nc.declare_dram_parameter("wk", [128, 4, 128], bf16, isOutput=False)
    d_vemb = nc.declare_dram_parameter("vemb", [128, 8, 128], bf16, isOutput=False)
    d_wv2 = nc.declare_dram_parameter("wv2", [128, 8, N], bf16, isOutput=False)
    d_bq = nc.declare_dram_parameter("bq", [128, 1], f32, isOutput=False)
    d_bk = nc.declare_dram_parameter("bk", [128, 1], f32, isOutput=False)
    d_bv2t = nc.declare_dram_parameter("bv2t", [128, 16], f32, isOutput=False)
    d_wout0 = nc.declare_dram_parameter("wout0", [64, D], bf16, isOutput=False)
    d_wout1 = nc.declare_dram_parameter("wout1", [64, D], bf16, isOutput=False)
    d_bmask = nc.declare_dram_parameter("bandmul", [128, 256], bf16, isOutput=False)
    d_bm1 = nc.declare_dram_parameter("bandm1", [128, 256], bf16, isOutput=False)

    d_attn = nc.declare_dram_parameter("attn", [2, N, N], f32, isOutput=True)
    d_pout = nc.declare_dram_parameter("pout", [N, D], f32, isOutput=True)

    def band_window(t):
        # global j-window of the band for row tile t, and the mask col slice
        if t == 0:
            return slice(0, 192), slice(64, 256)
        if t == NT - 1:
            return slice(N - 192, N), slice(0, 192)
        return slice(128 * t - 64, 128 * t + 192), slice(0, 256)

    def band_pieces(t):
        # pieces of the band window split at the 1024 psum-half boundary:
        # (half, cols-within-half slice, mask cols slice)
        w, mw = band_window(t)
        pieces = []
        for v in range(2):
            lo = max(w.start, 1024 * v)
            hi = min(w.stop, 1024 * (v + 1))
            if lo < hi:
                m0 = mw.start + (lo - w.start)
                pieces.append(
                    (v, slice(lo - 1024 * v, hi - 1024 * v),
                     slice(m0, m0 + (hi - lo)))
                )
        return pieces

    with tile.TileContext(nc) as tc:
        with (
            tc.tile_pool(name="singles", bufs=1) as singles,
            tc.tile_pool(name="mm", bufs=3, space="PSUM") as mm,
            tc.tile_pool(name="ovp", bufs=2, space="PSUM") as ovp,
            tc.tile_pool(name="etile", bufs=4) as etile,
            tc.tile_pool(name="atile", bufs=4) as atile,
            tc.tile_pool(name="otile", bufs=4) as otile,
        ):
            # ---- persistent SBUF tensors (q/k weights first: critical path) ----
            wq_sb = singles.tile([128, 4, 128], bf16)
            nc.sync.dma_start(out=wq_sb, in_=d_wq[:, :, :])
            wk_sb = singles.tile([128, 4, 128], bf16)
            nc.sync.dma_start(out=wk_sb, in_=d_wk[:, :, :])
            bq_sb = singles.tile([128, 1], f32)
            nc.sync.dma_start(out=bq_sb, in_=d_bq[:, :])
            bk_sb = singles.tile([128, 1], f32)
            nc.sync.dma_start(out=bk_sb, in_=d_bk[:, :])
            bv2t_sb = singles.tile([128, 16], f32)
            bmask_sb = singles.tile([128, 256], bf16)
            bm1_sb = singles.tile([128, 256], bf16)
            wout0_sb = singles.tile([64, D], bf16)
            wout1_sb = singles.tile([64, D], bf16)

            qpT_sb = singles.tile([128, N], bf16)  # d-pair on partitions
            kpT_sb = singles.tile([128, N], bf16)
            v_sb = singles.tile([128, 16, 128], bf16)
            ovacc = [
                singles.tile([64, N], bf16, tag=f"ovacc{h}", name=f"ovacc{h}")
                for h in range(2)
            ]
            recip = [
                singles.tile([128, 16], f32, tag=f"rc{h}", name=f"rc{h}")
                for h in range(2)
            ]

            # ---- projections ----
            with tc.tile_pool(name="inp", bufs=1) as inp:
                for d_src, w_sb, b_sb, dst in (
                    (d_qT, wq_sb, bq_sb, qpT_sb),
                    (d_kT, wk_sb, bk_sb, kpT_sb),
                ):
                    src_sb = inp.tile([128, 4, N], bf16, tag="src", name="src")
                    for kk in range(4):
                        nc.sync.dma_start(
                            out=src_sb[:, kk, :], in_=d_src[:, kk, :]
                        )
                    for n in range(NC):
                        ps = mm.tile([128, 512], f32, tag="mm", name="ps_prj")
                        for kk in range(4):
                            nc.tensor.matmul(
                                out=ps,
                                lhsT=w_sb[:, kk, :],
                                rhs=src_sb[:, kk, 512 * n : 512 * (n + 1)],
                                start=(kk == 0),
                                stop=(kk == 3),
                            )
                        nc.vector.tensor_scalar_add(
                            out=dst[:, 512 * n : 512 * (n + 1)], in0=ps, scalar1=b_sb
                        )

                nc.sync.dma_start(out=bmask_sb, in_=d_bmask[:, :])
                nc.sync.dma_start(out=bm1_sb, in_=d_bm1[:, :])
                nc.sync.dma_start(out=bv2t_sb, in_=d_bv2t[:, :])
                nc.sync.dma_start(out=wout0_sb, in_=d_wout0[:, :])
                nc.sync.dma_start(out=wout1_sb, in_=d_wout1[:, :])
                vemb_sb = singles.tile([128, 8, 128], bf16)
                nc.sync.dma_start(out=vemb_sb, in_=d_vemb[:, :, :])
                wv2_sb = singles.tile([128, 8, N], bf16)
                nc.sync.dma_start(out=wv2_sb, in_=d_wv2[:, :, :])

                def emit_vproj():
                    for t in range(16):
                        ps = mm.tile([128, 128], f32, tag="mm", name="ps_v")
                        for mk in range(8):
                            nc.tensor.matmul(
                                out=ps,
                                lhsT=wv2_sb[:, mk, 128 * t : 128 * (t + 1)],
                                rhs=vemb_sb[:, mk, :],
                                start=(mk == 0),
                                stop=(mk == 7),
                            )
                        nc.vector.tensor_scalar_add(
                            out=v_sb[:, t, :], in0=ps, scalar1=bv2t_sb[:, t : t + 1]
                        )

            U_sb = [
                singles.tile([128, 8, N], bf16, tag=f"U{h}", name=f"U{h}")
                for h in range(2)
            ]

            def emitA(h, t):
                """ST scores tile t -> exp -> U[:, t, :] -> band mask."""
                hr = slice(64 * h, 64 * h + 64)
                for v in range(2):
                    ps = mm.tile([128, 1024], f32, tag="mm", name=f"psA{h}_{t}_{v}")
                    for n in range(2):
                        nc.tensor.matmul(
                            out=ps[:, 512 * n : 512 * (n + 1)],
                            lhsT=kpT_sb[hr, 128 * t : 128 * (t + 1)],
                            rhs=qpT_sb[hr, 1024 * v + 512 * n : 1024 * v + 512 * (n + 1)],
                            start=True,
                            stop=True,
                        )
                    nc.scalar.activation(
                        out=U_sb[h][:, t % 8, 1024 * v : 1024 * (v + 1)],
                        in_=ps,
                        func=AF.Exp,
                    )
                w, mw = band_window(t)
                nc.vector.tensor_tensor(
                    out=U_sb[h][:, t % 8, w],
                    in0=U_sb[h][:, t % 8, w],
                    in1=bmask_sb[:, mw],
                    op=ALU.mult,
                )

            def emitOV(h, tg, half=None):
                """ov burst: U j-tiles of group tg x all i for head h."""
                ts = list(range(4 * tg, 4 * tg + 4))
                if half is not None:
                    ts = ts[2 * half : 2 * half + 2]
                for c in range(NC):
                    ps = ovp.tile(
                        [64, 512], f32, tag="ov", name=f"ps_ov{h}_{tg}_{half}_{c}"
                    )
                    for dt, t in enumerate(ts):
                        nc.tensor.matmul(
                            out=ps,
                            lhsT=v_sb[:, t, 64 * h : 64 * h + 64],
                            rhs=U_sb[h][:, t % 8, 512 * c : 512 * (c + 1)],
                            start=(dt == 0),
                            stop=(dt == len(ts) - 1),
                        )
                    dst = ovacc[h][:, 512 * c : 512 * (c + 1)]
                    if tg == 0:
                        nc.vector.tensor_copy(out=dst, in_=ps)
                    else:
                        nc.vector.tensor_tensor(
                            out=dst, in0=dst, in1=ps, op=ALU.add
                        )

            def emitB(h, t):
                """S scores tile t -> exp+rowsum -> band fix -> attn out."""
                hr = slice(64 * h, 64 * h + 64)
                e = etile.tile([128, N], bf16, tag="e", name=f"e{h}_{t}")
                rss = []
                for v in range(2):
                    ps = mm.tile(
                        [128, 1024], f32, tag="mm", name=f"psB{h}_{t}_{v}"
                    )
                    for n in range(2):
                        nc.tensor.matmul(
                            out=ps[:, 512 * n : 512 * (n + 1)],
                            lhsT=qpT_sb[hr, 128 * t : 128 * (t + 1)],
                            rhs=kpT_sb[
                                hr, 1024 * v + 512 * n : 1024 * v + 512 * (n + 1)
                            ],
                            start=True,
                            stop=True,
                        )
                    rs = otile.tile([128, 1], f32, tag="rs", name=f"rs{h}_{t}_{v}")
                    nc.scalar.activation(
                        out=e[:, 1024 * v : 1024 * (v + 1)], in_=ps,
                        func=AF.Exp, accum_out=rs,
                    )
                    rss.append(rs)
                # corrected rowsum (exclude band) + zero the band in e
                rs2 = otile.tile([128, 1], f32, tag="rs2", name=f"rs2{h}_{t}")
                nc.vector.tensor_tensor(
                    out=rs2, in0=rss[0], in1=rss[1], op=ALU.add
                )
                w, mw = band_window(t)
                scr = otile.tile([128, 256], bf16, tag="scr", name=f"scr{h}_{t}")
                nbs = otile.tile([128, 1], f32, tag="nbs", name=f"nbs{h}_{t}")
                nc.vector.tensor_tensor(
                    out=scr[:, mw], in0=e[:, w], in1=bm1_sb[:, mw], op=ALU.mult
                )
                nc.vector.tensor_reduce(
                    out=nbs, in_=scr[:, mw], axis=mybir.AxisListType.X, op=ALU.add
                )
                nc.vector.tensor_tensor(out=rs2, in0=rs2, in1=nbs, op=ALU.add)
                nc.vector.tensor_tensor(
                    out=e[:, w], in0=e[:, w], in1=bmask_sb[:, mw], op=ALU.mult
                )
                nc.vector.reciprocal(out=recip[h][:, t : t + 1], in_=rs2)
                at = atile.tile([128, N], f32, tag="at", name=f"at{h}_{t}")
                nc.vector.tensor_scalar_mul(
                    out=at, in0=e, scalar1=recip[h][:, t : t + 1]
                )
                nc.sync.dma_start(
                    out=d_attn[h, 128 * t : 128 * (t + 1), :], in_=at
                )

            # two-head interleaved emission: h0 score MMs run on PE array
            # rows 0-63 (tile T0), h1 on rows 64-127 (T8) -> concurrent
            for u in range(NT // 2):
                for t in (2 * u, 2 * u + 1):
                    emitA(0, t)
                    emitA(1, t)
                    emitB(0, t)
                    emitB(1, t)
                if u == 0:
                    emit_vproj()
                if u == 6:
                    emitOV(0, 3, half=0)
                    emitOV(1, 3, half=0)
                elif u == 7:
                    emitOV(0, 3, half=1)
                    emitOV(1, 3, half=1)
                elif u % 2 == 1:
                    emitOV(0, u // 2)
                    emitOV(1, u // 2)

            # ---- pout ----
            for t in range(NT):
                pps = []
                for h in range(2):
                    ps = mm.tile([128, 512], f32, tag="mm", name=f"ps_p{h}_{t}")
                    nc.tensor.matmul(
                        out=ps,
                        lhsT=ovacc[h][:, 128 * t : 128 * (t + 1)],
                        rhs=(wout0_sb if h == 0 else wout1_sb),
                        start=True,
                        stop=True,
                    )
                    pps.append(ps)
                tmp = otile.tile([128, 512], f32, tag="tmp", name=f"tmp{t}")
                nc.scalar.activation(
                    out=tmp,
                    in_=pps[1],
                    func=AF.Copy,
                    scale=recip[1][:, t : t + 1],
                )
                ot = otile.tile([128, 512], f32, tag="ot", name=f"ot{t}")
                nc.vector.scalar_tensor_tensor(
                    out=ot,
                    in0=pps[0],
                    scalar=recip[0][:, t : t + 1],
                    in1=tmp,
                    op0=ALU.mult,
                    op1=ALU.add,
                )
                nc.sync.dma_start(out=d_pout[128 * t : 128 * (t + 1), :], in_=ot)

    nc.compile()
    return nc


def _prep_shards(q, k, Wq, bq, Wk, bk, V_emb, Wv2, bv2, Wout):
    """Build the 8 per-core input maps (host-side sharding + bf16 cast)."""

    def b16(a):
        return np.ascontiguousarray(a.astype(BF16))

    # shared across cores
    wv2p = np.zeros((MPAD, N), np.float32)
    wv2p[:MAX_LEN] = Wv2
    wv2p = b16(wv2p.reshape(8, 128, N).transpose(1, 0, 2))
    bv2t = np.ascontiguousarray(bv2.reshape(16, 128).T.astype(np.float32))

    r = np.arange(128)[:, None]
    c = np.arange(256)[None, :]
    bandmul = np.where(np.abs(r + 64 - c) <= BAND, 0.0, 1.0).astype(BF16)
    bandm1 = (bandmul.astype(np.float32) - 1.0).astype(BF16)

    # per-batch
    qT = {}
    kT = {}
    for b in range(B):
        qT[b] = b16(q[b].T.reshape(4, 128, N).transpose(1, 0, 2))
        kT[b] = b16(k[b].T.reshape(4, 128, N).transpose(1, 0, 2))

    # per head-pair
    per_hp = {}
    for hp in range(4):
        cs = slice(128 * hp, 128 * (hp + 1))
        wq = b16((Wq[:, cs] / 8.0).reshape(4, 128, 128).transpose(1, 0, 2))
        wk = b16(Wk[:, cs].reshape(4, 128, 128).transpose(1, 0, 2))
        vemb = np.zeros((MPAD, 128), np.float32)
        vemb[:MAX_LEN] = V_emb[:, cs]
        vemb = b16(vemb.reshape(8, 128, 128).transpose(1, 0, 2))
        bqp = np.ascontiguousarray((bq[cs] / 8.0)[:, None].astype(np.float32))
        bkp = np.ascontiguousarray(bk[cs][:, None].astype(np.float32))
        wout0 = b16(Wout[128 * hp : 128 * hp + 64, :])
        wout1 = b16(Wout[128 * hp + 64 : 128 * (hp + 1), :])
        per_hp[hp] = (wq, wk, vemb, bqp, bkp, wout0, wout1)

    in_maps = []
    for core in range(8):
        b, hp = core // 4, core % 4
        wq, wk, vemb, bqp, bkp, wout0, wout1 = per_hp[hp]
        in_maps.append(
            {
                "qT": qT[b],
                "kT": kT[b],
                "wq": wq,
                "wk": wk,
                "vemb": vemb,
                "wv2": wv2p,
                "bq": bqp,
                "bk": bkp,
                "bv2t": bv2t,
                "wout0": wout0,
                "wout1": wout1,
                "bandmul": bandmul,
                "bandm1": bandm1,
            }
        )
    return in_maps


def kernel(q, k, v, Wq, bq, Wk, bk, V_emb, Wv2, bv2, Wout, bout):
    from concourse.bass_utils import run_bass_kernel_spmd

    q = np.asarray(q, np.float32)
    k = np.asarray(k, np.float32)
    Wq = np.asarray(Wq, np.float32)
    bq = np.asarray(bq, np.float32)
    Wk = np.asarray(Wk, np.float32)
    bk = np.asarray(bk, np.float32)
    V_emb = np.asarray(V_emb, np.float32)
    Wv2 = np.asarray(Wv2, np.float32)
    bv2 = np.asarray(bv2, np.float32)
    Wout = np.asarray(Wout, np.float32)
    bout = np.asarray(bout, np.float32)

    if "nc" not in _cache:
        _cache["nc"] = _build_nc()
    nc = _cache["nc"]

    in_maps = _prep_shards(q, k, Wq, bq, Wk, bk, V_emb, Wv2, bv2, Wout)
    res = run_bass_kernel_spmd(nc, in_maps, core_ids=list(range(8)))

    attn = np.empty((B, H, N, N), np.float32)
    out = np.empty((B, N, D), np.float32)
    for b in range(B):
        acc = np.zeros((N, D), np.float32)
        for hp in range(4):
            r = res.results[4 * b + hp]
            attn[b, 2 * hp : 2 * hp + 2] = r["attn"]
            acc += r["pout"]
        out[b] = acc + bout[None, :]
    return out, attn


# revision 22
# speedup vs baseline: 1.0335x; 1.0335x over previous
"""Distributed Trainium2 kernel for the sparse-attention module.

Shapes (hardcoded): B=2, N=2048, D=512, H=8, HD=64, MAX_LEN=1000, BAND=3.
Sharding: 16 (batch, head) pairs over 8 cores -> each core owns one batch b
and one head-pair hp (2 heads = 128 cols of the projected D dimension).

Per core (all matmuls bf16, f32 accumulation):
  qpT = (Wq_cols/8)^T @ q[b]^T   [128, 2048]   (1/8 score scale folded in)
  kpT = Wk_cols^T @ k[b]^T       [128, 2048]
  v   = Wv2^T @ V_emb_cols       [2048, 128]   (s on partitions, 16 tiles)
  per head h (rows 64h:64h+64 of qpT/kpT), two independent pipelines:
   A: ST scores (j on part.) -> band -> exp -> U bf16; ov += v_h^T @ U (psum
      bursts of 4 j-tiles, reduced into ovacc bf16 in SBUF)
   B: S scores (i on part.) -> band -> exp(accum_out=rowsum) -> recip ->
      normalize on VectorE -> DMA attn out
  pout = sum_h (ovacc_h^T @ Wout_h) * recip_h[i]  (host sums across cores)
"""

import sys
import os
import numpy as np

sys.path.insert(0, "/opt/trn_rl_repo")

import ml_dtypes

BF16 = ml_dtypes.bfloat16
FP8 = ml_dtypes.float8_e4m3

B, N, D, H = 2, 2048, 512, 8
HD = D // H
MAX_LEN = 1000
MPAD = 1024  # V_emb/Wv2 contraction dim padded to 8*128
BAND = 3
NT = N // 128  # 16 row tiles
NC = N // 512  # 4 free chunks

_cache = {}


def _build_nc():
    import concourse.bacc as bacc
    import concourse.tile as tile
    from concourse import mybir

    f32 = mybir.dt.float32
    bf16 = mybir.dt.bfloat16
    fp8 = mybir.dt.float8e4
    AF = mybir.ActivationFunctionType
    ALU = mybir.AluOpType

    nc = bacc.Bacc("TRN2", target_bir_lowering=False, num_devices=8)

    # ---- DRAM parameters (per-core shards; host preps layouts) ----
    d_qT = nc.declare_dram_parameter("qT", [128, 4, N], bf16, isOutput=False)
    d_kT = nc.declare_dram_parameter("kT", [128, 4, N], bf16, isOutput=False)
    d_wq = nc.declare_dram_parameter("wq", [128, 4, 128], bf16, isOutput=False)
    d_wk = nc.declare_dram_parameter("wk", [128, 4, 128], bf16, isOutput=False)
    d_vemb = nc.declare_dram_parameter("vemb", [128, 8, 128], bf16, isOutput=False)
    d_wv2 = nc.declare_dram_parameter("wv2", [128, 8, N], bf16, isOutput=False)
    d_bq = nc.declare_dram_parameter("bq", [128, 1], f32, isOutput=False)
    d_bk = nc.declare_dram_parameter("bk", [128, 1], f32, isOutput=False)
    d_bv2t = nc.declare_dram_parameter("bv2t", [128, 16], f32, isOutput=False)
    d_wout0 = nc.declare_dram_parameter("wout0", [64, D], bf16, isOutput=False)
    d_wout1 = nc.declare_dram_parameter("wout1", [64, D], bf16, isOutput=False)
    d_bmask = nc.declare_dram_parameter("bandmul", [128, 256], bf16, isOutput=False)
    d_bm1 = nc.declare_dram_parameter("bandm1", [128, 256], bf16, isOutput=False)

    d_attn = nc.declare_dram_parameter("attn", [2, N, N], f32, isOutput=True)
    d_pout = nc.declare_dram_parameter("pout", [N, D], f32, isOutput=True)

    def band_window(t):
        # global j-window of the band for row tile t, and the mask col slice
        if t == 0:
            return slice(0, 192), slice(64, 256)
        if t == NT - 1:
            return slice(N - 192, N), slice(0, 192)
        return slice(128 * t - 64, 128 * t + 192), slice(0, 256)

    def band_pieces(t):
        # pieces of the band window split at the 1024 psum-half boundary:
        # (half, cols-within-half slice, mask cols slice)
        w, mw = band_window(t)
        pieces = []
        for v in range(2):
            lo = max(w.start, 1024 * v)
            hi = min(w.stop, 1024 * (v + 1))
            if lo < hi:
                m0 = mw.start + (lo - w.start)
                pieces.append(
                    (v, slice(lo - 1024 * v, hi - 1024 * v),
                     slice(m0, m0 + (hi - lo)))
                )
        return pieces

    with tile.TileContext(nc) as tc:
        with (
            tc.tile_pool(name="singles", bufs=1) as singles,
            tc.tile_pool(name="mm", bufs=3, space="PSUM") as mm,
            tc.tile_pool(name="ovp", bufs=2, space="PSUM") as ovp,
            tc.tile_pool(name="etile", bufs=3) as etile,
            tc.tile_pool(name="atile", bufs=3) as atile,
            tc.tile_pool(name="otile", bufs=4) as otile,
        ):
            # ---- persistent SBUF tensors (q/k weights first: critical path) ----
            wq_sb = singles.tile([128, 4, 128], bf16)
            nc.sync.dma_start(out=wq_sb, in_=d_wq[:, :, :])
            wk_sb = singles.tile([128, 4, 128], bf16)
            nc.sync.dma_start(out=wk_sb, in_=d_wk[:, :, :])
            bq_sb = singles.tile([128, 1], f32)
            nc.sync.dma_start(out=bq_sb, in_=d_bq[:, :])
            bk_sb = singles.tile([128, 1], f32)
            nc.sync.dma_start(out=bk_sb, in_=d_bk[:, :])
            bv2t_sb = singles.tile([128, 16], f32)
            bmask_sb = singles.tile([128, 256], bf16)
            bm1_sb = singles.tile([128, 256], bf16)
            wout0_sb = singles.tile([64, D], bf16)
            wout1_sb = singles.tile([64, D], bf16)

            qpT_sb = singles.tile([128, N], bf16)  # d-pair on partitions
            kpT_sb = singles.tile([128, N], bf16)
            v_sb = singles.tile([128, 16, 128], bf16)
            ovacc = [
                singles.tile([64, N], bf16, tag=f"ovacc{h}", name=f"ovacc{h}")
                for h in range(2)
            ]
            recip = [
                singles.tile([128, 16], f32, tag=f"rc{h}", name=f"rc{h}")
                for h in range(2)
            ]

            # ---- projections ----
            with tc.tile_pool(name="inp", bufs=1) as inp:
                for d_src, w_sb, b_sb, dst in (
                    (d_qT, wq_sb, bq_sb, qpT_sb),
                    (d_kT, wk_sb, bk_sb, kpT_sb),
                ):
                    src_sb = inp.tile([128, 4, N], bf16, tag="src", name="src")
                    for kk in range(4):
                        nc.sync.dma_start(
                            out=src_sb[:, kk, :], in_=d_src[:, kk, :]
                        )
                    for n in range(NC):
                        ps = mm.tile([128, 512], f32, tag="mm", name="ps_prj")
                        for kk in range(4):
                            nc.tensor.matmul(
                                out=ps,
                                lhsT=w_sb[:, kk, :],
                                rhs=src_sb[:, kk, 512 * n : 512 * (n + 1)],
                                start=(kk == 0),
                                stop=(kk == 3),
                            )
                        nc.vector.tensor_scalar_add(
                            out=dst[:, 512 * n : 512 * (n + 1)], in0=ps, scalar1=b_sb
                        )

                nc.sync.dma_start(out=bmask_sb, in_=d_bmask[:, :])
                nc.sync.dma_start(out=bm1_sb, in_=d_bm1[:, :])
                nc.sync.dma_start(out=bv2t_sb, in_=d_bv2t[:, :])
                nc.sync.dma_start(out=wout0_sb, in_=d_wout0[:, :])
                nc.sync.dma_start(out=wout1_sb, in_=d_wout1[:, :])
                vemb_sb = singles.tile([128, 8, 128], bf16)
                nc.sync.dma_start(out=vemb_sb, in_=d_vemb[:, :, :])
                wv2_sb = singles.tile([128, 8, N], bf16)
                nc.sync.dma_start(out=wv2_sb, in_=d_wv2[:, :, :])

                def emit_vproj():
                    for t in range(16):
                        ps = mm.tile([128, 128], f32, tag="mm", name="ps_v")
                        for mk in range(8):
                            nc.tensor.matmul(
                                out=ps,
                                lhsT=wv2_sb[:, mk, 128 * t : 128 * (t + 1)],
                                rhs=vemb_sb[:, mk, :],
                                start=(mk == 0),
                                stop=(mk == 7),
                            )
                        nc.vector.tensor_scalar_add(
                            out=v_sb[:, t, :], in0=ps, scalar1=bv2t_sb[:, t : t + 1]
                        )

            U_sb = [
                singles.tile([128, 8, N], bf16, tag=f"U{h}", name=f"U{h}")
                for h in range(2)
            ]

            def emitA(h, t):
                """ST scores tile t -> exp -> U[:, t, :] -> band mask."""
                hr = slice(64 * h, 64 * h + 64)
                for v in range(2):
                    ps = mm.tile([128, 1024], f32, tag="mm", name=f"psA{h}_{t}_{v}")
                    for n in range(2):
                        nc.tensor.matmul(
                            out=ps[:, 512 * n : 512 * (n + 1)],
                            lhsT=kpT_sb[hr, 128 * t : 128 * (t + 1)],
                            rhs=qpT_sb[hr, 1024 * v + 512 * n : 1024 * v + 512 * (n + 1)],
                            start=True,
                            stop=True,
                        )
                    nc.scalar.activation(
                        out=U_sb[h][:, t % 8, 1024 * v : 1024 * (v + 1)],
                        in_=ps,
                        func=AF.Exp,
                    )
                w, mw = band_window(t)
                nc.vector.tensor_tensor(
                    out=U_sb[h][:, t % 8, w],
                    in0=U_sb[h][:, t % 8, w],
                    in1=bmask_sb[:, mw],
                    op=ALU.mult,
                )

            def emitOV(h, tg, half=None):
                """ov burst: U j-tiles of group tg x all i for head h."""
                ts = list(range(4 * tg, 4 * tg + 4))
                if half is not None:
                    ts = ts[2 * half : 2 * half + 2]
                for c in range(NC):
                    ps = ovp.tile(
                        [64, 512], f32, tag="ov", name=f"ps_ov{h}_{tg}_{half}_{c}"
                    )
                    for dt, t in enumerate(ts):
                        nc.tensor.matmul(
                            out=ps,
                            lhsT=v_sb[:, t, 64 * h : 64 * h + 64],
                            rhs=U_sb[h][:, t % 8, 512 * c : 512 * (c + 1)],
                            start=(dt == 0),
                            stop=(dt == len(ts) - 1),
                        )
                    dst = ovacc[h][:, 512 * c : 512 * (c + 1)]
                    if tg == 0:
                        nc.vector.tensor_copy(out=dst, in_=ps)
                    else:
                        nc.vector.tensor_tensor(
                            out=dst, in0=dst, in1=ps, op=ALU.add
                        )

            def emitB(h, t):
                """S scores tile t -> exp+rowsum -> band fix -> attn out."""
                hr = slice(64 * h, 64 * h + 64)
                e = etile.tile([128, N], bf16, tag="e", name=f"e{h}_{t}")
                rss = []
                for v in range(2):
                    ps = mm.tile(
                        [128, 1024], f32, tag="mm", name=f"psB{h}_{t}_{v}"
                    )
                    for n in range(2):
                        nc.tensor.matmul(
                            out=ps[:, 512 * n : 512 * (n + 1)],
                            lhsT=qpT_sb[hr, 128 * t : 128 * (t + 1)],
                            rhs=kpT_sb[
                                hr, 1024 * v + 512 * n : 1024 * v + 512 * (n + 1)
                            ],
                            start=True,
                            stop=True,
                        )
                    rs = otile.tile([128, 1], f32, tag="rs", name=f"rs{h}_{t}_{v}")
                    nc.scalar.activation(
                        out=e[:, 1024 * v : 1024 * (v + 1)], in_=ps,
                        func=AF.Exp, accum_out=rs,
                    )
                    rss.append(rs)
                # corrected rowsum (exclude band) + zero the band in e
                rs2 = otile.tile([128, 1], f32, tag="rs2", name=f"rs2{h}_{t}")
                nc.vector.tensor_tensor(
                    out=rs2, in0=rss[0], in1=rss[1], op=ALU.add
                )
                w, mw = band_window(t)
                scr = otile.tile([128, 256], bf16, tag="scr", name=f"scr{h}_{t}")
                nbs = otile.tile([128, 1], f32, tag="nbs", name=f"nbs{h}_{t}")
                nc.vector.tensor_tensor(
                    out=scr[:, mw], in0=e[:, w], in1=bm1_sb[:, mw], op=ALU.mult
                )
                nc.vector.tensor_reduce(
                    out=nbs, in_=scr[:, mw], axis=mybir.AxisListType.X, op=ALU.add
                )
                nc.vector.tensor_tensor(out=rs2, in0=rs2, in1=nbs, op=ALU.add)
                nc.vector.tensor_tensor(
                    out=e[:, w], in0=e[:, w], in1=bmask_sb[:, mw], op=ALU.mult
                )
                nc.vector.reciprocal(out=recip[h][:, t : t + 1], in_=rs2)
                at = atile.tile([128, N], f32, tag="at", name=f"at{h}_{t}")
                nc.vector.tensor_scalar_mul(
                    out=at, in0=e, scalar1=recip[h][:, t : t + 1]
                )
                nc.sync.dma_start(
                    out=d_attn[h, 128 * t : 128 * (t + 1), :], in_=at
                )

            # two-head interleaved emission: h0 score MMs run on PE array
            # rows 0-63 (tile T0), h1 on rows 64-127 (T8) -> concurrent
            for u in range(NT // 2):
                for t in (2 * u, 2 * u + 1):
                    emitA(0, t)
                    emitA(1, t)
                    emitB(0, t)
                    emitB(1, t)
                if u == 0:
                    emit_vproj()
                if u == 6:
                    emitOV(0, 3, half=0)
                    emitOV(1, 3, half=0)
                elif u == 7:
                    emitOV(0, 3, half=1)
                    emitOV(1, 3, half=1)
                elif u % 2 == 1:
                    emitOV(0, u // 2)
                    emitOV(1, u // 2)

            # ---- pout ----
            for t in range(NT):
                pps = []
                for h in range(2):
                    ps = mm.tile([128, 512], f32, tag="mm", name=f"ps_p{h}_{t}")
                    nc.tensor.matmul(
                        out=ps,
                        lhsT=ovacc[h][:, 128 * t : 128 * (t + 1)],
                        rhs=(wout0_sb if h == 0 else wout1_sb),
                        start=True,
                        stop=True,
                    )
                    pps.append(ps)
                tmp = otile.tile([128, 512], f32, tag="tmp", name=f"tmp{t}")
                nc.scalar.activation(
                    out=tmp,
                    in_=pps[1],
                    func=AF.Copy,
                    scale=recip[1][:, t : t + 1],
                )
                ot = otile.tile([128, 512], f32, tag="ot", name=f"ot{t}")
                nc.vector.scalar_tensor_tensor(
                    out=ot,
                    in0=pps[0],
                    scalar=recip[0][:, t : t + 1],
                    in1=tmp,
                    op0=ALU.mult,
                    op1=ALU.add,
                )
                nc.sync.dma_start(out=d_pout[128 * t : 128 * (t + 1), :], in_=ot)

    nc.compile()
    return nc


def _prep_shards(q, k, Wq, bq, Wk, bk, V_emb, Wv2, bv2, Wout):
    """Build the 8 per-core input maps (host-side sharding + bf16 cast)."""

    def b16(a):
        return np.ascontiguousarray(a.astype(BF16))

    # shared across cores
    wv2p = np.zeros((MPAD, N), np.float32)
    wv2p[:MAX_LEN] = Wv2
    wv2p = b16(wv2p.reshape(8, 128, N).transpose(1, 0, 2))
    bv2t = np.ascontiguousarray(bv2.reshape(16, 128).T.astype(np.float32))

    r = np.arange(128)[:, None]
    c = np.arange(256)[None, :]
    bandmul = np.where(np.abs(r + 64 - c) <= BAND, 0.0, 1.0).astype(BF16)
    bandm1 = (bandmul.astype(np.float32) - 1.0).astype(BF16)

    # per-batch
    qT = {}
    kT = {}
    for b in range(B):
        qT[b] = b16(q[b].T.reshape(4, 128, N).transpose(1, 0, 2))
        kT[b] = b16(k[b].T.reshape(4, 128, N).transpose(1, 0, 2))

    # per head-pair
    per_hp = {}
    for hp in range(4):
        cs = slice(128 * hp, 128 * (hp + 1))
        wq = b16((Wq[:, cs] / 8.0).reshape(4, 128, 128).transpose(1, 0, 2))
        wk = b16(Wk[:, cs].reshape(4, 128, 128).transpose(1, 0, 2))
        vemb = np.zeros((MPAD, 128), np.float32)
        vemb[:MAX_LEN] = V_emb[:, cs]
        vemb = b16(vemb.reshape(8, 128, 128).transpose(1, 0, 2))
        bqp = np.ascontiguousarray((bq[cs] / 8.0)[:, None].astype(np.float32))
        bkp = np.ascontiguousarray(bk[cs][:, None].astype(np.float32))
        wout0 = b16(Wout[128 * hp : 128 * hp + 64, :])
        wout1 = b16(Wout[128 * hp + 64 : 128 * (hp + 1), :])
        per_hp[hp] = (wq, wk, vemb, bqp, bkp, wout0, wout1)

    in_maps = []
    for core in range(8):
        b, hp = core // 4, core % 4
        wq, wk, vemb, bqp, bkp, wout0, wout1 = per_hp[hp]
        in_maps.append(
            {
                "qT": qT[b],
                "kT": kT[b],
                "wq": wq,
                "wk": wk,
                "vemb": vemb,
                "wv2": wv2p,
                "bq": bqp,
                "bk": bkp,
                "bv2t": bv2t,
                "wout0": wout0,
                "wout1": wout1,
                "bandmul": bandmul,
                "bandm1": bandm1,
            }
        )
    return in_maps


def kernel(q, k, v, Wq, bq, Wk, bk, V_emb, Wv2, bv2, Wout, bout):
    from concourse.bass_utils import run_bass_kernel_spmd

    q = np.asarray(q, np.float32)
    k = np.asarray(k, np.float32)
    Wq = np.asarray(Wq, np.float32)
    bq = np.asarray(bq, np.float32)
    Wk = np.asarray(Wk, np.float32)
    bk = np.asarray(bk, np.float32)
    V_emb = np.asarray(V_emb, np.float32)
    Wv2 = np.asarray(Wv2, np.float32)
    bv2 = np.asarray(bv2, np.float32)
    Wout = np.asarray(Wout, np.float32)
    bout = np.asarray(bout, np.float32)

    if "nc" not in _cache:
        _cache["nc"] = _build_nc()
    nc = _cache["nc"]

    in_maps = _prep_shards(q, k, Wq, bq, Wk, bk, V_emb, Wv2, bv2, Wout)
    res = run_bass_kernel_spmd(nc, in_maps, core_ids=list(range(8)))

    attn = np.empty((B, H, N, N), np.float32)
    out = np.empty((B, N, D), np.float32)
    for b in range(B):
        acc = np.zeros((N, D), np.float32)
        for hp in range(4):
            r = res.results[4 * b + hp]
            attn[b, 2 * hp : 2 * hp + 2] = r["attn"]
            acc += r["pout"]
        out[b] = acc + bout[None, :]
    return out, attn
